# revision 5
# baseline (speedup 1.0000x reference)
"""Multi-LoRA batched low-rank adapter kernel for 8 trn2 NeuronCores.

Problem: x [16, 2048, 4096] f32, adapter_ids [16] int, A [64, 4096, 64],
B [64, 64, 4096].  out[b] = (x[b] @ B[id_b].T) @ A[id_b].T * (1/64).

Sharding: data-parallel over batch (2 samples/core); per-sample
adapters are gathered on host (adapter_ids are host-visible and tiny)
and x is pre-transposed + seq-blocked on host so the mm1 contraction
dim lands on SBUF partitions and every DMA is a large contiguous read.

Numerics: the correctness gate is rel_err < 2e-2, so everything runs
as a SINGLE bf16 term with fp32 PSUM accumulation (simulated rel err
~3.9e-3 vs fp64 incl. bf16 output rounding; measured 4.2e-3 on HW).
This is 3x fewer PE matmuls than a hi/lo split and half the DMA bytes
(x hi only, out stored bf16 and upcast on host).

Structure: the sample is processed in 4 seq-blocks of 512.  Per block:
load x^T block [4096, 512] (4x 1MB quarter-DMAs so mm1 starts after
the first MB), 32 accumulating mm1 matmuls into one PSUM bank, one
DVE drain to bf16, then 32 mm2 matmuls (stationary = Bx chunk, moving
= A^T) drained DVE/ACT-alternating into a bf16 staging tile, stored
as 2MB transfers on gpsimd/SWDGE.  mm2 of block g is emitted after
mm1 of block g+1 so the bx drain latency hides under PE work.  This
keeps the software pipeline full ~10us after launch (vs ~45us for a
whole-sample mm1/mm2 split) which matters for single-shot exec time.

Rank (64) is zero-padded to 128 so both matmuls keep K=128 (fast
weight load path; K=64 measured +200ns/matmul on this HW).
"""

import numpy as np
from contextlib import ExitStack

import concourse.bass as bass
import concourse.tile as tile
from concourse import bacc, mybir, bass_utils

NCORES = 8
BATCH = 16
B_PER = BATCH // NCORES
SEQ = 2048
DIN = 4096
DOUT = 4096
RANK = 64
RPAD = 128
SCALE = np.float32(1.0 / 64.0)

f32 = mybir.dt.float32
bf16 = mybir.dt.bfloat16

P = 128
KI = DIN // P       # 32 contraction tiles for mm1
SB = 512            # seq block
NBLK = SEQ // SB    # 4
NSB = SB // P       # 4 output row-chunks per block
OT = DOUT // 512    # 8
XQ = 4              # x quarter-DMAs per block
KQ = KI // XQ       # 8 k-tiles per quarter

_CACHE = {}


def _build_nc(repeat=1):
    nc = bacc.Bacc("TRN2", target_bir_lowering=False, debug=False)
    xb_d = nc.dram_tensor("xb", [B_PER, NBLK, DIN, SB], bf16,
                          kind="ExternalInput").ap()
    bh_d = nc.dram_tensor("bh", [B_PER, DIN, RPAD], bf16,
                          kind="ExternalInput").ap()
    ah_d = nc.dram_tensor("ah", [B_PER, RPAD, DOUT], bf16,
                          kind="ExternalInput").ap()
    out = nc.dram_tensor("out", [B_PER, SEQ, DOUT], bf16,
                         kind="ExternalOutput").ap()

    with tile.TileContext(nc) as tc, ExitStack() as ctx:
        adp = ctx.enter_context(tc.tile_pool(name="adp", bufs=2))
        xbp = ctx.enter_context(tc.tile_pool(name="xbp", bufs=2))
        bxsp = ctx.enter_context(tc.tile_pool(name="bxsp", bufs=2))
        stg = ctx.enter_context(tc.tile_pool(name="stg", bufs=2))
        bxps = ctx.enter_context(tc.tile_pool(name="bxps", bufs=2, space="PSUM"))
        outp = ctx.enter_context(tc.tile_pool(name="outp", bufs=3, space="PSUM"))

        def load_adapters(s):
            ad = {}
            t = adp.tile([P, KI, RPAD], bf16, name="bh", tag="bh")
            for q in range(XQ):
                nc.sync.dma_start(
                    t[:, q * KQ:(q + 1) * KQ, :],
                    bh_d[s, q * KQ * P:(q + 1) * KQ * P, :].rearrange(
                        "(k p) r -> p k r", p=P))
            ad["bh"] = t
            t = adp.tile([RPAD, DOUT], bf16, name="ah", tag="ah")
            nc.sync.dma_start(t[:], ah_d[s])
            ad["ah"] = t
            return ad

        def mm1_block(s, blk, ad):
            """Load x block (4 quarter-DMAs) + 32 accumulating matmuls,
            drain to bf16."""
            xt = xbp.tile([P, KI, SB], bf16, name="xt", tag="xt")
            for q in range(XQ):
                nc.sync.dma_start(
                    xt[:, q * KQ:(q + 1) * KQ, :],
                    xb_d[s, blk, q * KQ * P:(q + 1) * KQ * P, :].rearrange(
                        "(k p) m -> p k m", p=P))
            bx = bxps.tile([P, SB], f32, name="bx", tag="bx")
            for k in range(KI):
                nc.tensor.matmul(bx[:], ad["bh"][:, k, :], xt[:, k, :],
                                 start=(k == 0), stop=(k == KI - 1))
            bxh = bxsp.tile([RPAD, SB], bf16, name="bxh", tag="bxh")
            nc.vector.tensor_copy(bxh[:], bx[:])
            return bxh

        def mm2_block(s, blk, ad, bxh, fine_store):
            st = stg.tile([P, NSB, DOUT], bf16, name="st", tag="st")
            for ns in range(NSB):
                for otp in range(OT // 2):
                    ps = outp.tile([P, 1024], f32, name="ps_o", tag="ps_o")
                    for half in range(2):
                        ot = otp * 2 + half
                        ov = slice(ot * 512, (ot + 1) * 512)
                        pv = slice(half * 512, (half + 1) * 512)
                        nc.tensor.matmul(ps[:, pv], bxh[:, ns * P:(ns + 1) * P],
                                         ad["ah"][:, ov], start=True, stop=True)
                    dv = slice(otp * 1024, (otp + 1) * 1024)
                    if otp % 2 == 0:
                        nc.vector.tensor_copy(st[:, ns, dv], ps[:])
                    else:
                        nc.scalar.copy(st[:, ns, dv], ps[:])
                if fine_store:
                    nc.gpsimd.dma_start(
                        out[s, blk * SB + ns * P: blk * SB + (ns + 1) * P, :],
                        st[:, ns, :])
            if not fine_store:
                for h in range(2):
                    r0 = blk * SB + h * 2 * P
                    nc.gpsimd.dma_start(
                        out[s, r0:r0 + 2 * P, :].rearrange(
                            "(f p) n -> p f n", p=P),
                        st[:, 2 * h:2 * h + 2, :])

        samples = [s for _ in range(repeat) for s in range(B_PER)]
        blocks = [(s, blk) for s in samples for blk in range(NBLK)]

        ad_cur = load_adapters(samples[0])
        ads = [ad_cur] + [None] * (len(samples) - 1)
        prev = None
        for g, (s, blk) in enumerate(blocks):
            spos = g // NBLK
            # prefetch next sample's adapters mid-way through this sample
            if blk == 2 and spos + 1 < len(samples):
                ads[spos + 1] = load_adapters(samples[spos + 1])
            bxh = mm1_block(s, blk, ads[spos])
            if prev is not None:
                mm2_block(*prev)
            prev = (s, blk, ads[spos], bxh, g == len(blocks) - 1)
        mm2_block(*prev)
    nc.compile()
    return nc


def _get_nc(repeat=1):
    key = f"nc{repeat}"
    if key not in _CACHE:
        _CACHE[key] = _build_nc(repeat)
    return _CACHE[key]


def _prep_in_maps(x, adapter_ids, A, B):
    import ml_dtypes
    x = np.asarray(x, dtype=np.float32)
    ids = np.asarray(adapter_ids).astype(np.int64)
    A = np.asarray(A, dtype=np.float32)
    B = np.asarray(B, dtype=np.float32)

    As = A * SCALE
    in_maps = []
    for c in range(NCORES):
        sl = slice(c * B_PER, (c + 1) * B_PER)
        cids = ids[sl]
        xT = x[sl].transpose(0, 2, 1)                       # [2, DIN, SEQ]
        xb = np.ascontiguousarray(
            xT.reshape(B_PER, DIN, NBLK, SB).transpose(0, 2, 1, 3))
        BT = np.zeros((B_PER, DIN, RPAD), np.float32)
        BT[:, :, :RANK] = B[cids].transpose(0, 2, 1)
        AT = np.zeros((B_PER, RPAD, DOUT), np.float32)
        AT[:, :RANK, :] = As[cids].transpose(0, 2, 1)
        in_maps.append({
            "xb": xb.astype(ml_dtypes.bfloat16),
            "bh": BT.astype(ml_dtypes.bfloat16),
            "ah": AT.astype(ml_dtypes.bfloat16),
        })
    return in_maps


def kernel(x, adapter_ids, A, B):
    nc = _get_nc()
    in_maps = _prep_in_maps(x, adapter_ids, A, B)
    res = bass_utils.run_bass_kernel_spmd(
        nc, in_maps, core_ids=list(range(NCORES)))
    out = np.empty((BATCH, SEQ, DOUT), dtype=np.float32)
    for c in range(NCORES):
        out[c * B_PER:(c + 1) * B_PER] = res.results[c]["out"].astype(np.float32)
    return out
